# revision 5
# baseline (speedup 1.0000x reference)
"""Trainium2 Bass kernel for nn_DechunkingLayer (ragged_sequence).

Reference semantics (per batch row):
    idx = clip(exclusive_cumsum(b), 0, NC - 1)          # [T]
    up[t]  = z[idx[t]]                                  # gather rows
    out[t] = p[t] * up[t] + (1 - p[t]) * up[t-1]        # EMA blend
    out[0] = up[0]

Sharding: pure data parallel over batch B=8 across the 8 NeuronCores
(one batch row per core).

v3 design (baseline was PE-bound at ~2.5us/tile on a full-tile fp32
shift matmul):
  - permuted tile layout: partition p = 32c + r holds t = 128k + 4r + c.
    Then up[t-1] sits at partition p-32 for 3/4 of rows, so `rolled` is
    a quadrant-aligned partition shift — legal on DVE (bank0->any
    quadrant for <=32-wide ops; 64-wide writes to [64:128) allowed).
  - rolled production split: PE shifts only the LEFT half-tile columns
    (one [128,512] fp32 matmul with a permuted-shift 0/1 matrix); DVE
    copies the right half (two cross-quadrant copies); the 32 hard rows
    (t % 4 == 0, whose predecessor is 95 partitions away) get their
    right half via a tiny partition-shifted SBUF->SBUF DMA.
  - final store in bf16 (half the store traffic). Rounding the FINAL
    value is relative-error-safe (<= 2^-9) even under cancellation;
    rounding any blend INPUT would not be.
  - rows t = 128k blend against the previous tile's last row; redone
    exactly in the same epilogue pass as the baseline (same HWDGE
    queue, FIFO overwrite).
  - out[0] = up[0] exactly via forcing p[0] = 1.
"""

import numpy as np

import concourse.bacc as bacc
import concourse.bass as bass
import concourse.mybir as mybir
import concourse.tile as tile
from concourse.bass import IndirectOffsetOnAxis
from concourse.bass_utils import run_bass_kernel_spmd
from concourse.masks import make_identity, make_upper_triangular

# Problem shape (hardcoded per harness contract).
B = 8          # batch rows == number of cores
T = 4096       # timesteps per row
NCH = 2048     # number of chunks (z rows)
D = 1024       # d_model
P = 128        # SBUF partitions
NT = T // P    # 32 tiles per core
NCOL = T // P  # 32 columns in the W layout
DH = D // 2    # matmul free-dim max for fp32 is 512

F32 = mybir.dt.float32
BF16 = mybir.dt.bfloat16
I32 = mybir.dt.int32

WARMUP_MM = 8  # PE warm-up matmuls to release the HAM clock throttle


def build_bass() -> bass.Bass:
    # Bacc (not raw Bass): its finalize() runs generate_event_semaphores,
    # which splits multi-sem waits to satisfy TRN2's one-wait-per-instruction
    # ISA constraint.
    nc = bacc.Bacc()

    z = nc.dram_tensor("z", [NCH, D], F32, kind="ExternalInput")
    p = nc.dram_tensor("p", [T], F32, kind="ExternalInput")
    b = nc.dram_tensor("b", [T], I32, kind="ExternalInput")
    out = nc.dram_tensor("out", [T, D], BF16, kind="ExternalOutput")

    with tile.TileContext(nc) as tc:
        with (
            tc.tile_pool(name="setup", bufs=1) as sp,
            tc.tile_pool(name="psmall", bufs=2, space="PSUM") as pps,
            tc.tile_pool(name="proll", bufs=3, space="PSUM") as ppr,
            tc.tile_pool(name="main", bufs=5) as mp,
        ):
            # ---- constants -------------------------------------------------
            # affine_select only exists on gpsimd; PE Matmult has a single
            # sync-wait slot, so launder every matmul operand through DVE so
            # all matmul waits collapse onto one DVE semaphore.
            tri_g = sp.tile([P, P], F32)     # tri[k, i] = 1 iff i > k
            make_upper_triangular(nc, tri_g[:], val=1.0, diag=False)
            tri = sp.tile([P, P], F32)
            nc.vector.tensor_copy(out=tri[:], in_=tri_g[:])

            ident_g = sp.tile([NCOL, NCOL], F32)
            make_identity(nc, ident_g[:])
            ident = sp.tile([NCOL, NCOL], F32)
            nc.vector.tensor_copy(out=ident[:], in_=ident_g[:])

            tri32_g = sp.tile([NCOL, NCOL], F32)  # [k, j] = 1 iff j > k
            make_upper_triangular(nc, tri32_g[:], val=1.0, diag=False)
            tri32 = sp.tile([NCOL, NCOL], F32)
            nc.vector.tensor_copy(out=tri32[:], in_=tri32_g[:])

            # P permutation matrix: perm[a, i] = 1 iff a == sigma(i),
            # sigma(32c + r) = 4r + c. matmul(lhsT=perm, rhs=x)[i] = x[sigma(i)].
            # Built as the 128-identity with columns re-ordered via a
            # strided-AP copy: column (c, r) reads identity column 4r + c.
            id128_g = sp.tile([P, P], F32)
            make_identity(nc, id128_g[:])
            perm = sp.tile([P, P], F32)
            nc.vector.tensor_copy(
                out=perm[:].rearrange("a (c r) -> a c r", c=4, r=32),
                in_=id128_g[:].rearrange("a (r c) -> a c r", r=32, c=4),
            )

            # Permuted shift matrix: sperm[j, i] = 1 iff j == src(i);
            # src(i) = i - 32 for i >= 32, src(i) = 95 + i for 1 <= i < 32,
            # column 0 all-zero (row t=128k fixed by the epilogue).
            sh_g = sp.tile([P, P], F32)
            nc.gpsimd.memset(sh_g[:], 0.0)
            # band i = k + 32 on the full matrix (only lands in cols >= 32)
            nc.gpsimd.affine_select(
                out=sh_g[:], in_=sh_g[:],
                compare_op=mybir.AluOpType.not_equal, fill=1.0,
                base=32, pattern=[[-1, P]], channel_multiplier=1,
            )
            # band i = k - 95, restricted to cols [0:32); k in [96, 126]
            nc.gpsimd.affine_select(
                out=sh_g[:, 0:32], in_=sh_g[:, 0:32],
                compare_op=mybir.AluOpType.not_equal, fill=1.0,
                base=-95, pattern=[[-1, 32]], channel_multiplier=1,
            )
            # clear the stray hit at (k=127, i=32) the first band did not
            # create but k-95 would at i=32 if pattern leaked; also ensure
            # column 0 is zero. (Band 2 on sliced cols cannot touch i>=32.)
            sperm = sp.tile([P, P], F32)
            nc.vector.tensor_copy(out=sperm[:], in_=sh_g[:])

            ones_row = sp.tile([1, P], F32)  # lhsT for partition-broadcast
            nc.vector.memset(ones_row[:], 1.0)
            ones_col = sp.tile([P, 1], F32)  # lhsT for column sums
            nc.vector.memset(ones_col[:], 1.0)

            # ---- load b and p in natural [32, 128] layout ------------------
            b2d = b[:].rearrange("(j c) -> j c", c=P)          # [32, 128] DRAM view
            p2d = p[:].rearrange("(j c) -> j c", c=P)

            b_nat_i = sp.tile([NCOL, P], I32)
            nc.sync.dma_start(out=b_nat_i[:], in_=b2d)
            p_nat = sp.tile([NCOL, P], F32)
            nc.sync.dma_start(out=p_nat[:], in_=p2d)

            b_nat = sp.tile([NCOL, P], F32)
            nc.vector.tensor_copy(out=b_nat[:], in_=b_nat_i[:])

            # ---- PE transpose to W layout [128, 32]: (p, j) = t = 128j + p --
            bw_ps = pps.tile([P, NCOL], F32, space="PSUM", tag="small_ps")
            nc.tensor.transpose(out=bw_ps[:], in_=b_nat[:], identity=ident[:])
            b_w = sp.tile([P, NCOL], F32)
            nc.vector.tensor_copy(out=b_w[:], in_=bw_ps[:])

            # tile-0 indices on a short path: colofs[0] = 0, so column 0
            # needs only the partition scan — the first gather can issue
            # before the column-offset chain finishes.
            s0_ps = pps.tile([P, 1], F32, space="PSUM", tag="small_ps")
            nc.tensor.matmul(out=s0_ps[:], lhsT=tri[:], rhs=b_w[:, 0:1],
                             start=True, stop=True)
            idx0_f = sp.tile([P, 1], F32)
            nc.vector.tensor_scalar_min(out=idx0_f[:], in0=s0_ps[:],
                                        scalar1=float(NCH - 1))
            g0_ps = pps.tile([P, 1], F32, space="PSUM", tag="small_ps")
            nc.tensor.matmul(out=g0_ps[:], lhsT=perm[:], rhs=idx0_f[:],
                             start=True, stop=True)
            idxg0_i = sp.tile([P, 1], I32)
            nc.vector.tensor_copy(out=idxg0_i[:], in_=g0_ps[:])

            pw_ps = pps.tile([P, NCOL], F32, space="PSUM", tag="small_ps")
            nc.tensor.transpose(out=pw_ps[:], in_=p_nat[:], identity=ident[:])
            p_w = sp.tile([P, NCOL], F32)
            nc.vector.tensor_copy(out=p_w[:], in_=pw_ps[:])
            # out[0] = up[0] exactly: force p[0] = 1 so the blend is 1*up + 0*rolled
            nc.vector.memset(p_w[0:1, 0:1], 1.0)
            q_w = sp.tile([P, NCOL], F32)  # q = 1 - p (std layout, for epilogue)
            nc.scalar.activation(
                out=q_w[:], in_=p_w[:],
                func=mybir.ActivationFunctionType.Copy, bias=1.0, scale=-1.0,
            )

            # permuted p / q for the main loop
            pg_ps = pps.tile([P, NCOL], F32, space="PSUM", tag="small_ps")
            nc.tensor.matmul(out=pg_ps[:], lhsT=perm[:], rhs=p_w[:],
                             start=True, stop=True)
            p_g = sp.tile([P, NCOL], F32)
            nc.vector.tensor_copy(out=p_g[:], in_=pg_ps[:])
            q_g = sp.tile([P, NCOL], F32)
            nc.scalar.activation(
                out=q_g[:], in_=p_g[:],
                func=mybir.ActivationFunctionType.Copy, bias=1.0, scale=-1.0,
            )

            # ---- column offsets via two PE matmuls -------------------------
            totc_ps = pps.tile([NCOL, 1], F32, space="PSUM", tag="small_ps")
            nc.tensor.matmul(out=totc_ps[:], lhsT=b_w[:], rhs=ones_col[:],
                             start=True, stop=True)
            tot_col = sp.tile([NCOL, 1], F32)
            nc.vector.tensor_copy(out=tot_col[:], in_=totc_ps[:])
            cofs_ps = pps.tile([1, NCOL], F32, space="PSUM", tag="small_ps")
            nc.tensor.matmul(out=cofs_ps[:], lhsT=tot_col[:], rhs=tri32[:],
                             start=True, stop=True)
            colofs = sp.tile([1, NCOL], F32)
            nc.vector.tensor_copy(out=colofs[:], in_=cofs_ps[:])

            # ---- full exclusive cumsum s[t] in W layout --------------------
            s_ps = pps.tile([P, NCOL], F32, space="PSUM", tag="small_ps")
            nc.tensor.matmul(out=s_ps[:], lhsT=tri[:], rhs=b_w[:],
                             start=True, stop=False)
            nc.tensor.matmul(out=s_ps[:], lhsT=ones_row[:], rhs=colofs[:],
                             start=False, stop=True)

            # ---- gather indices: idx = min(s, NCH-1), std + permuted -------
            idx_f = sp.tile([P, NCOL], F32)
            nc.vector.tensor_scalar_min(out=idx_f[:], in0=s_ps[:], scalar1=float(NCH - 1))
            gi_ps = pps.tile([P, NCOL], F32, space="PSUM", tag="small_ps")
            nc.tensor.matmul(out=gi_ps[:], lhsT=perm[:], rhs=idx_f[:],
                             start=True, stop=True)
            idxg_i = sp.tile([P, NCOL], I32)
            nc.vector.tensor_copy(out=idxg_i[:], in_=gi_ps[:])

            # ---- epilogue vectors for rows t = 128j ------------------------
            # bprev_row[j] = idx[128j - 1] (0 for j=0, harmless: q[0]=0).
            bprev_row = sp.tile([1, NCOL], F32)
            nc.vector.memset(bprev_row[:], 0.0)
            nc.sync.dma_start(
                out=bprev_row[0:1, 1:NCOL], in_=idx_f[P - 1 : P, 0 : NCOL - 1]
            )

            cols_ps = pps.tile([NCOL, 4], F32, space="PSUM", tag="small_ps")
            for ci, row in enumerate([bprev_row, idx_f, p_w, q_w]):
                nc.tensor.matmul(
                    out=cols_ps[:, ci : ci + 1],
                    lhsT=row[0:1, 0:NCOL],
                    rhs=ones_row[0:1, 0:1],
                    start=True, stop=True,
                )
            bidx_i = sp.tile([NCOL, 1], I32)
            nc.vector.tensor_copy(out=bidx_i[:], in_=cols_ps[:, 0:1])
            fidx_i = sp.tile([NCOL, 1], I32)
            nc.vector.tensor_copy(out=fidx_i[:], in_=cols_ps[:, 1:2])
            pb_col = sp.tile([NCOL, 1], F32)
            nc.vector.tensor_copy(out=pb_col[:], in_=cols_ps[:, 2:3])
            qb_col = sp.tile([NCOL, 1], F32)
            nc.vector.tensor_copy(out=qb_col[:], in_=cols_ps[:, 3:4])

            # PE warm-up against the HAM clock throttle, burned in while the
            # first gather is in flight.
            warm_src = sp.tile([P, DH], F32)
            nc.vector.memset(warm_src[:], 1.0)
            for w in range(WARMUP_MM):
                wps = ppr.tile([P, DH], F32, space="PSUM", tag="roll")
                nc.tensor.matmul(out=wps[:], lhsT=sperm[:], rhs=warm_src[:],
                                 start=True, stop=True)
                if w == WARMUP_MM - 1:
                    warm_sink = sp.tile([1, 1], F32)
                    nc.vector.tensor_copy(out=warm_sink[:], in_=wps[0:1, 0:1])

            # store view: row t = 128k + 4r + c <- partition 32c + r
            out_v = out[:].rearrange("(k r c) d -> k c r d", r=32, c=4)

            # ---- main loop: gather, roll, blend, store ---------------------
            for k in range(NT):
                up = mp.tile([P, D], F32, tag="up")
                idx_col = idxg0_i[:, 0:1] if k == 0 else idxg_i[:, k : k + 1]
                nc.gpsimd.indirect_dma_start(
                    out=up[:], out_offset=None, in_=z[:],
                    in_offset=IndirectOffsetOnAxis(ap=idx_col, axis=0),
                )

                # rolled left half on PE: rps[i] = up[src(i)], cols [0:DH)
                rps = ppr.tile([P, DH], F32, space="PSUM", tag="roll")
                nc.tensor.matmul(out=rps[:], lhsT=sperm[:], rhs=up[:, 0:DH],
                                 start=True, stop=True)

                # rolled right half: DVE cross-quadrant copies + tiny DMA
                rr = mp.tile([P, DH], F32, tag="rr")
                nc.vector.tensor_copy(out=rr[32:64, :], in_=up[0:32, DH:D])
                nc.vector.tensor_copy(out=rr[64:96, :], in_=up[32:64, DH:D])
                nc.vector.tensor_copy(out=rr[96:128, :], in_=up[64:96, DH:D])
                # rows [0:32): src partitions [95:127) (row 0 gets junk;
                # t = 128k is redone by the epilogue)
                nc.scalar.dma_start(out=rr[0:32, :], in_=up[95:127, DH:D])

                # t1 = p * up on ACT
                t1 = mp.tile([P, D], F32, tag="t1")
                nc.scalar.mul(out=t1[:], in_=up[:], mul=p_g[:, k : k + 1])

                # o = (rolled * q) + t1 on DVE, bf16 out
                o = mp.tile([P, D], BF16, tag="o")
                nc.vector.scalar_tensor_tensor(
                    out=o[:, 0:DH], in0=rps[:], scalar=q_g[:, k : k + 1],
                    in1=t1[:, 0:DH],
                    op0=mybir.AluOpType.mult, op1=mybir.AluOpType.add,
                )
                nc.vector.scalar_tensor_tensor(
                    out=o[:, DH:D], in0=rr[:], scalar=q_g[:, k : k + 1],
                    in1=t1[:, DH:D],
                    op0=mybir.AluOpType.mult, op1=mybir.AluOpType.add,
                )

                # permuted store: 4 strided stores, one per quadrant
                for c in range(4):
                    nc.sync.dma_start(
                        out=out_v[k : k + 1, c : c + 1],
                        in_=o[32 * c : 32 * (c + 1), :],
                    )

                if k == 8:
                    # epilogue gathers + blend for rows t = 128j, issued
                    # mid-loop to ride gather-stream slack.
                    upf = sp.tile([NCOL, D], F32)
                    nc.gpsimd.indirect_dma_start(
                        out=upf[:], out_offset=None, in_=z[:],
                        in_offset=IndirectOffsetOnAxis(ap=fidx_i[:, 0:1], axis=0),
                    )
                    rollf = sp.tile([NCOL, D], F32)
                    nc.gpsimd.indirect_dma_start(
                        out=rollf[:], out_offset=None, in_=z[:],
                        in_offset=IndirectOffsetOnAxis(ap=bidx_i[:, 0:1], axis=0),
                    )
                    t1b = sp.tile([NCOL, D], F32)
                    nc.scalar.mul(out=t1b[:], in_=upf[:], mul=pb_col[:])
                    ob = sp.tile([NCOL, D], BF16)
                    nc.vector.scalar_tensor_tensor(
                        out=ob[:], in0=rollf[:], scalar=qb_col[:], in1=t1b[:],
                        op0=mybir.AluOpType.mult, op1=mybir.AluOpType.add,
                    )

            # ---- epilogue store: redo rows t = 128j exactly ----------------
            # Same HWDGE queue as the main stores, so FIFO order makes this
            # overwrite win.
            out_rows0 = out[:].rearrange("(j r) d -> j r d", r=P)[:, 0:1, :]
            nc.sync.dma_start(out=out_rows0, in_=ob[:, None, :])

    nc.finalize()
    return nc


_NC_CACHE = None


def _get_nc() -> bass.Bass:
    global _NC_CACHE
    if _NC_CACHE is None:
        _NC_CACHE = build_bass()
    return _NC_CACHE


def make_in_maps(z: np.ndarray, p: np.ndarray, b: np.ndarray) -> list[dict]:
    return [
        {
            "z": np.ascontiguousarray(z[i], dtype=np.float32),
            "p": np.ascontiguousarray(p[i], dtype=np.float32),
            "b": np.ascontiguousarray(b[i], dtype=np.int32),
        }
        for i in range(B)
    ]


def kernel(z, p, b, original_len=None, **_unused) -> np.ndarray:
    z = np.asarray(z, dtype=np.float32)
    p = np.asarray(p, dtype=np.float32)
    b = np.asarray(b, dtype=np.int32)
    assert z.shape == (B, NCH, D) and p.shape == (B, T) and b.shape == (B, T)

    nc = _get_nc()
    res = run_bass_kernel_spmd(nc, make_in_maps(z, p, b), list(range(B)))
    return np.stack(
        [np.asarray(r["out"]).astype(np.float32) for r in res.results], axis=0
    )
